# revision 13
# baseline (speedup 1.0000x reference)
"""Trainium2 Bass kernel for LoRA causal self-attention (GPT-style block).

Problem: B=4, T=2048, C=1024, H=16 heads, d=64, LoRA rank 8.
reference returns (out, query, key) where
  qkv  = x @ Wa^T + ba + (x @ Aa^T) @ Ba^T
  att  = causal softmax(q k^T / sqrt(d))
  y    = att @ v
  out  = y @ Wp^T + bp + (y @ Ap^T) @ Bp^T

Sharding: 8 cores = (batch b in 0..3) x (head-group g in 0..1, 8 heads each).
Per core the device computes, for its (b, g):
  - qT,kT = (Wqk_eff @ x_b^T) + bias    [feature-major, 512+512 x 2048]
  - v     = x_b @ Wv_eff^T + bias       [token-major, 2048 x 512]
  - per head: S^T = k q^T, exp(S/8) with causal min-mask, y^T = v^T-aug matmul
  - out^T partial = Wp_sub^T.T-style matmul over this core's 512 y-features
LoRA is folded into the weights host-side (exact here since B matrices are
zero), qkv biases are applied on device, proj bias host-side. The host
transposes/concats per-core outputs and sums the two partial out products
per batch.

All matmuls run as float32r (4-byte fp32 streamed at bf16 rate for moving
dim >= 256 on trn2); tiles are fp32 and bitcast at the matmul call sites.
"""

import numpy as np

B, T, C, H, D = 4, 2048, 1024, 16, 64
NCORES = 8
HPC = H // 2          # heads per core (head-group of 8)
GF = HPC * D          # features per head-group = 512
TQ = 512              # query tile
KBLK = 128            # key block
XCH = 256             # x token chunk for stage A
NEG = -1.0e30
POS = 3.0e38

_CACHE = {}


def _legalize_waits(nc, mybir):
    """This walrus build rejects any instruction with >1 sync wait; hoist
    extra waits onto single-wait NoOps on the same engine."""
    for fn in nc.m.functions:
        for blk in fn.blocks:
            new_insts = []
            changed = False
            for inst in blk.instructions:
                si = inst.sync_info
                if si is not None and si.on_wait and len(si.on_wait) > 1:
                    for w in si.on_wait:
                        nop = mybir.InstNoOp(
                            name=nc.get_next_instruction_name(),
                            engine=inst.engine,
                            bass_nofuse=True,
                            sync_info=mybir.SyncInfo(on_wait=[w], on_update=[]),
                        )
                        new_insts.append(nop)
                    inst.sync_info = mybir.SyncInfo(
                        on_wait=[], on_update=list(si.on_update)
                    )
                    changed = True
                new_insts.append(inst)
            if changed:
                blk.instructions = new_insts


def _build_nc():
    import concourse.bass as bass
    import concourse.mybir as mybir
    import concourse.tile as tile
    from contextlib import ExitStack

    f32 = mybir.dt.float32
    f32r = mybir.dt.float32r
    Exp = mybir.ActivationFunctionType.Exp

    def r(ap):
        return ap

    nc = bass.Bass()

    xT = nc.declare_dram_parameter("xT", [C, T], f32r, isOutput=False)
    wqk = nc.declare_dram_parameter("wqk", [C, 2 * GF], f32r, isOutput=False)
    wv = nc.declare_dram_parameter("wv", [C, GF], f32r, isOutput=False)
    wp = nc.declare_dram_parameter("wp", [GF, C], f32r, isOutput=False)
    bqk = nc.declare_dram_parameter("bqk", [128, 8], f32, isOutput=False)
    bv = nc.declare_dram_parameter("bv", [GF], f32, isOutput=False)
    mneg = nc.declare_dram_parameter("mneg", [128, 128], f32, isOutput=False)
    q_t = nc.declare_dram_parameter("q_t", [GF, T], f32r, isOutput=True)
    k_t = nc.declare_dram_parameter("k_t", [GF, T], f32r, isOutput=True)
    o_t = nc.declare_dram_parameter("o_t", [C, T], f32, isOutput=True)

    NCH = T // XCH  # 8 chunks
    with tile.TileContext(nc) as tc, ExitStack() as ctx:
        p_const = ctx.enter_context(tc.tile_pool(name="const", bufs=1))
        p_yT = ctx.enter_context(tc.tile_pool(name="yT", bufs=1))

        mneg_sb = p_const.tile([128, 128], f32, tag="mneg", name="mneg_sb")
        nc.sync.dma_start(out=mneg_sb[:], in_=mneg[:])
        bv_sb = p_const.tile([128, GF], f32, tag="bv", name="bv_sb")
        nc.sync.dma_start(out=bv_sb[:], in_=bv[None, :].to_broadcast([128, GF]))
        bqk_sb = p_const.tile([128, 8], f32, tag="bqk", name="bqk_sb")
        nc.sync.dma_start(out=bqk_sb[:], in_=bqk[:])
        ones_sb = p_const.tile([128, HPC], f32, tag="ones", name="ones_sb")
        nc.vector.memset(ones_sb[:], 1.0)

        yT_sb = [p_yT.tile([128, T], f32r, tag=f"y{j}", name=f"yT{j}") for j in range(4)]

        with (
            tc.tile_pool(name="qk", bufs=1) as p_qk,
            tc.tile_pool(name="v", bufs=1) as p_v,
        ):
            qk_sb = [p_qk.tile([128, T], f32r, tag=f"qk{f}", name=f"qk_sb{f}") for f in range(8)]
            # v tiles: per 128-token block, 8 heads x 65 cols.
            # even head h: cols [65h..65h+63] = v, col 65h+64 = 1.0
            # odd head h: col 65h = 1.0, cols [65h+1..65h+64] = v
            v_sb = [p_v.tile([128, HPC * 65], f32r, tag=f"v{i}", name=f"v_sb{i}") for i in range(T // KBLK)]

            # ---------------- stage A: projections ----------------
            with (
                tc.tile_pool(name="w", bufs=1) as p_w,
                tc.tile_pool(name="x", bufs=2) as p_x,
                tc.tile_pool(name="psA", bufs=2, space="PSUM") as psA,
            ):
                wqk_sb = [p_w.tile([128, 2 * GF], f32r, tag=f"wqk{c}", name=f"wqk_sb{c}") for c in range(8)]
                for c in range(8):
                    nc.sync.dma_start(out=wqk_sb[c][:], in_=wqk[c * 128:(c + 1) * 128, :])
                wv_sb = [p_w.tile([128, GF], f32r, tag=f"wv{c}", name=f"wv_sb{c}") for c in range(8)]
                for c in range(8):
                    nc.sync.dma_start(out=wv_sb[c][:], in_=wv[c * 128:(c + 1) * 128, :])

                for ch in range(NCH):
                    t0 = ch * XCH
                    xs = []
                    for c in range(8):
                        xt = p_x.tile([128, XCH], f32r, tag=f"x{c}", name=f"xt{c}")
                        nc.sync.dma_start(
                            out=xt[:], in_=xT[c * 128:(c + 1) * 128, t0:t0 + XCH]
                        )
                        xs.append(xt)
                    # q,k features (feature-major): psum[f-tile, tok]
                    for f in range(8):
                        ps = psA.tile([128, XCH], f32, tag="qkps", name="qkps")
                        for c in range(8):
                            nc.tensor.matmul(
                                ps[:],
                                r(wqk_sb[c][:, f * 128:(f + 1) * 128]),
                                r(xs[c][:]),
                                start=(c == 0),
                                stop=(c == 7),
                            )
                        nc.vector.tensor_scalar_add(
                            out=qk_sb[f][:, t0:t0 + XCH],
                            in0=ps[:],
                            scalar1=bqk_sb[:, f:f + 1],
                        )
                    # v (token-major): psum[tok-subtile, feat]
                    for sub in range(XCH // KBLK):
                        pv = psA.tile([128, GF], f32, tag="vps", name="vps")
                        for c in range(8):
                            nc.tensor.matmul(
                                pv[:],
                                r(xs[c][:, sub * 128:(sub + 1) * 128]),
                                r(wv_sb[c][:]),
                                start=(c == 0),
                                stop=(c == 7),
                            )
                        ti = ch * (XCH // KBLK) + sub
                        vt = v_sb[ti].rearrange("p (h e) -> p h e", e=65)
                        pvv = pv.rearrange("p (h e) -> p h e", e=64)
                        bvv = bv_sb.rearrange("p (h e) -> p h e", e=64)
                        nc.vector.tensor_add(
                            out=vt[:, :, 0:64], in0=pvv[:], in1=bvv[:]
                        )
                        nc.vector.tensor_copy(
                            out=vt[:, :, 64:65],
                            in_=ones_sb.rearrange("p (h e) -> p h e", e=1),
                        )

            # write q,k outputs (feature-major; host transposes)
            for f in range(4):
                nc.sync.dma_start(out=q_t[f * 128:(f + 1) * 128, :], in_=qk_sb[f][:])
                nc.sync.dma_start(out=k_t[f * 128:(f + 1) * 128, :], in_=qk_sb[4 + f][:])

            # ---------------- stage B: attention ----------------
            with (
                tc.tile_pool(name="att", bufs=2) as p_att,
                tc.tile_pool(name="sm", bufs=2) as p_sm,
                tc.tile_pool(name="dscr", bufs=2, space="DRAM") as p_dscr,
                tc.tile_pool(name="psS", bufs=2, space="PSUM") as psS,
                tc.tile_pool(name="psY", bufs=1, space="PSUM") as psY,
            ):
                for hp in range(4):
                    qtile = qk_sb[hp]
                    ktile = qk_sb[4 + hp]
                    for qt in range(4):
                        yps = [
                            psY.tile([128, TQ], f32, tag=f"y{hi}", name=f"yps{hi}") for hi in range(2)
                        ]
                        nkb = 4 * qt + 4
                        for kb in range(nkb):
                            j = kb - 4 * qt  # >= 0 on diagonal blocks
                            col0 = max(0, j) * 128
                            for hi in range(2):
                                row0 = hi * 64
                                sps = psS.tile([128, TQ], f32, tag=f"s{hi}", name=f"sps{hi}")
                                nc.tensor.matmul(
                                    sps[:, col0:TQ],
                                    r(ktile[row0:row0 + 64, kb * 128:(kb + 1) * 128]),
                                    r(qtile[row0:row0 + 64, qt * TQ + col0:(qt + 1) * TQ]),
                                    start=True,
                                    stop=True,
                                )
                                if j >= 0:
                                    nc.vector.tensor_tensor(
                                        out=sps[:, col0:col0 + 128],
                                        in0=sps[:, col0:col0 + 128],
                                        in1=mneg_sb[:],
                                        op=mybir.AluOpType.min,
                                    )
                                att = p_att.tile([128, TQ], f32r, tag=f"att{hi}", name=f"att{hi}")
                                nc.scalar.activation(
                                    out=att[:, col0:TQ],
                                    in_=sps[:, col0:TQ],
                                    func=Exp,
                                    scale=0.125,
                                )
                                h = 2 * hp + hi
                                v65 = v_sb[kb][:, h * 65:h * 65 + 65]
                                # rows 0..63 = y, row 64 = softmax sum
                                nc.tensor.matmul(
                                    yps[hi][0:65, col0:TQ],
                                    r(v65),
                                    r(att[:, col0:TQ]),
                                    start=(kb == 0),
                                    stop=(kb == nkb - 1),
                                )
                        # normalize y and store to yT (feature-major)
                        for hi in range(2):
                            ysrc = yps[hi]
                            rec = p_sm.tile([128, TQ], f32, tag="rec", name="rec")
                            nc.vector.reciprocal(
                                out=rec[64:65, :], in_=ysrc[64:65, :]
                            )
                            # broadcast row 64 -> rows 0..63 via DRAM bounce
                            # (SBUF->SBUF partition-broadcast DMA is illegal)
                            dscr = p_dscr.tile([1, TQ], f32, tag="dscr", name="dscr")
                            nc.sync.dma_start(out=dscr[:], in_=rec[64:65, :])
                            nc.sync.dma_start(
                                out=rec[0:64, :], in_=dscr[:].to_broadcast([64, TQ])
                            )
                            if hi == 0:
                                nc.vector.tensor_mul(
                                    out=yT_sb[hp][0:64, qt * TQ:(qt + 1) * TQ],
                                    in0=ysrc[0:64, :],
                                    in1=rec[0:64, :],
                                )
                            else:
                                tmp = p_sm.tile([128, TQ], f32r, tag="tmp", name="tmp")
                                nc.vector.tensor_mul(
                                    out=tmp[0:64, :],
                                    in0=ysrc[0:64, :],
                                    in1=rec[0:64, :],
                                )
                                nc.sync.dma_start(
                                    out=yT_sb[hp][64:128, qt * TQ:(qt + 1) * TQ],
                                    in_=tmp[0:64, :],
                                )

        # ---------------- stage C: output projection (partial) ----------------
        with (
            tc.tile_pool(name="wp", bufs=1) as p_wp,
            tc.tile_pool(name="og", bufs=3) as p_og,
            tc.tile_pool(name="psC", bufs=3, space="PSUM") as psC,
        ):
            wp_sb = [p_wp.tile([128, C], f32r, tag=f"wp{j}", name=f"wp_sb{j}") for j in range(4)]
            for j in range(4):
                nc.sync.dma_start(out=wp_sb[j][:], in_=wp[j * 128:(j + 1) * 128, :])
            for ot in range(8):
                for tch in range(4):
                    pp = psC.tile([128, TQ], f32, tag="pp", name="pp")
                    for j in range(4):
                        nc.tensor.matmul(
                            pp[:],
                            r(wp_sb[j][:, ot * 128:(ot + 1) * 128]),
                            r(yT_sb[j][:, tch * TQ:(tch + 1) * TQ]),
                            start=(j == 0),
                            stop=(j == 3),
                        )
                    og = p_og.tile([128, TQ], f32, tag="og", name="og")
                    nc.vector.tensor_copy(out=og[:], in_=pp[:])
                    nc.sync.dma_start(
                        out=o_t[ot * 128:(ot + 1) * 128, tch * TQ:(tch + 1) * TQ],
                        in_=og[:],
                    )

    _legalize_waits(nc, mybir)
    return nc


def get_nc():
    if "nc" not in _CACHE:
        _CACHE["nc"] = _build_nc()
    return _CACHE["nc"]


def make_in_maps(x, Wa_eff, ba, Wp_eff):
    """Build the 8 per-core input maps from full tensors."""
    mneg = np.where(
        np.arange(128)[None, :] >= np.arange(128)[:, None], POS, NEG
    ).astype(np.float32)
    in_maps = []
    for core in range(NCORES):
        b, g = core // 2, core % 2
        sl = slice(g * GF, (g + 1) * GF)
        wq = Wa_eff[0:C][sl]
        wk = Wa_eff[C:2 * C][sl]
        wvm = Wa_eff[2 * C:3 * C][sl]
        bq = ba[0:C][sl]
        bk = ba[C:2 * C][sl]
        bvv = ba[2 * C:3 * C][sl]
        in_maps.append({
            "xT": np.ascontiguousarray(x[b].T),
            "wqk": np.ascontiguousarray(np.concatenate([wq, wk], axis=0).T),
            "wv": np.ascontiguousarray(wvm.T),
            "wp": np.ascontiguousarray(Wp_eff[:, sl].T),
            "bqk": np.ascontiguousarray(
                np.concatenate([bq, bk]).reshape(8, 128).T
            ),
            "bv": np.ascontiguousarray(bvv),
            "mneg": mneg,
        })
    return in_maps


def assemble(results, bp):
    """Combine per-core outputs into (out, query, key)."""
    query = np.empty((B, T, C), np.float32)
    key = np.empty((B, T, C), np.float32)
    out = np.zeros((B, T, C), np.float32)
    for core in range(NCORES):
        b, g = core // 2, core % 2
        sl = slice(g * GF, (g + 1) * GF)
        r = results[core]
        query[b, :, sl] = r["q_t"].T
        key[b, :, sl] = r["k_t"].T
        out[b] += r["o_t"].T
    out += bp[None, None, :]
    return out, query, key


def kernel(**inputs):
    from concourse.bass_utils import run_bass_kernel_spmd

    x = np.asarray(inputs["x"], np.float32)
    Wa = np.asarray(inputs["c_attn_w"], np.float32)
    ba = np.asarray(inputs["c_attn_b"], np.float32)
    Aa = np.asarray(inputs["c_attn_A"], np.float32)
    Ba = np.asarray(inputs["c_attn_B"], np.float32)
    Wp = np.asarray(inputs["c_proj_w"], np.float32)
    bp = np.asarray(inputs["c_proj_b"], np.float32)
    Ap = np.asarray(inputs["c_proj_A"], np.float32)
    Bp = np.asarray(inputs["c_proj_B"], np.float32)
    n_head = int(np.asarray(inputs["n_head"]))
    assert n_head == H and x.shape == (B, T, C)

    Wa_eff = Wa + Ba.astype(np.float64) @ Aa.astype(np.float64)
    Wa_eff = Wa_eff.astype(np.float32)
    Wp_eff = Wp + Bp.astype(np.float64) @ Ap.astype(np.float64)
    Wp_eff = Wp_eff.astype(np.float32)

    nc = get_nc()
    in_maps = make_in_maps(x, Wa_eff, ba, Wp_eff)
    res = run_bass_kernel_spmd(nc, in_maps, core_ids=list(range(NCORES)))
    return assemble(res.results, bp)
